# revision 1
# baseline (speedup 1.0000x reference)
"""Causal self-attention (B=2, T=2048, dim=2048, H=16, D=128) on 8 trn2 NeuronCores.

Sharding: data-parallel over batch (2 groups of 4 cores), tensor-parallel over
heads within a group (4 heads/core).  Each core computes its heads' QKV
projection (x @ Wqkv_part^T), RoPE, causal attention, and a partial output
projection against its W_proj column block; the host sums the 4 partials per
batch element.

Device layouts (all matmul operands bf16, fp32 accumulation):
  - x is passed transposed (dim, T) so QKV comes out head-transposed:
      qT/kT per head: (d=128 partitions, T free)  -> S^T = kT.T @ qT directly
  - v in natural (T, d) layout (projection computed with swapped operands)
  - attention: S^T tiles (128 keys x <=512 queries), exp on ScalarE from PSUM,
    causal handled by trimming fully-masked tiles + a triangular mask multiply
    on the diagonal 128x128 block; softmax normalization deferred past PV
    (rowsums via ones-matmul, broadcast via gpsimd, one fused multiply).
"""

import os

import numpy as np
import ml_dtypes

import concourse.bass as bass
import concourse.bacc as bacc
import concourse.tile as tile
import concourse.mybir as mybir
from concourse import bass_utils

BF16 = mybir.dt.bfloat16
F32 = mybir.dt.float32

B, T, DIM = 2, 2048, 2048
H, D = 16, 128
HL = 4                   # heads per core
NCORES = 8
E = 3 * HL * D           # 1536 = per-core qkv output rows
NCHUNK = DIM // 128      # 16 contraction chunks
NW = T // 512            # 4 query windows
NTT = T // 128           # 16 token tiles
SCALE = 1.0 / float(np.sqrt(D))

_CACHE = {}
LAST_RESULTS = None


def _build_module(loop_iters=1):
    nc = bacc.Bacc("TRN2", target_bir_lowering=False, debug=False)
    xT = nc.dram_tensor("xT", (DIM, T), BF16, kind="ExternalInput")
    wqkvT = nc.dram_tensor("wqkvT", (DIM, E), BF16, kind="ExternalInput")
    wpT = nc.dram_tensor("wpT", (HL * D, DIM), BF16, kind="ExternalInput")
    cosT = nc.dram_tensor("cosT", (D, T), F32, kind="ExternalInput")
    sinTs = nc.dram_tensor("sinTs", (D, T), F32, kind="ExternalInput")
    tri = nc.dram_tensor("tri", (128, 128), BF16, kind="ExternalInput")
    ones = nc.dram_tensor("ones", (128, 1), BF16, kind="ExternalInput")
    ones_row = nc.dram_tensor("ones_row", (1, 128), F32, kind="ExternalInput")
    y = nc.dram_tensor("y", (T, DIM), F32, kind="ExternalOutput")

    Exp = mybir.ActivationFunctionType.Exp

    with tile.TileContext(nc) as tc:
        with (
            tc.tile_pool(name="const", bufs=1) as cpool,
            tc.tile_pool(name="xp", bufs=2) as xpool,
            tc.tile_pool(name="rotp", bufs=2) as rotpool,
            tc.tile_pool(name="ptp", bufs=4) as ptpool,
            tc.tile_pool(name="yp", bufs=4) as ypool,
            tc.tile_pool(name="bcp", bufs=2) as bcpool,
            tc.tile_pool(name="rcpp", bufs=2) as rcppool,
            tc.tile_pool(name="ps", bufs=6, space="PSUM") as pspool,
            tc.tile_pool(name="rsps", bufs=2, space="PSUM") as rspool,
        ):
            cos_sb = cpool.tile([128, T], F32, tag="cos")
            nc.sync.dma_start(cos_sb[:], cosT[:, :])
            sin_sb = cpool.tile([128, T], F32, tag="sin")
            nc.sync.dma_start(sin_sb[:], sinTs[:, :])
            tri_sb = cpool.tile([128, 128], BF16, tag="tri")
            nc.sync.dma_start(tri_sb[:], tri[:, :])
            ones_sb = cpool.tile([128, 1], BF16, tag="ones")
            nc.sync.dma_start(ones_sb[:], ones[:, :])
            onesr_sb = cpool.tile([1, 128], F32, tag="onesr")
            nc.sync.dma_start(onesr_sb[:], ones_row[:, :])
            w_sb = cpool.tile([128, NCHUNK, E], BF16, tag="w")
            nc.sync.dma_start(w_sb[:], wqkvT.rearrange("(c p) e -> p c e", p=128))
            wp_sb = cpool.tile([128, HL, DIM], BF16, tag="wp")
            nc.sync.dma_start(wp_sb[:], wpT.rearrange("(h p) n -> p h n", p=128))

            q_sb = cpool.tile([128, HL * T], BF16, tag="q")
            k_sb = cpool.tile([128, HL * T], BF16, tag="k")
            v_sb = cpool.tile([128, NTT * HL * D], BF16, tag="v")
            o_sb = cpool.tile([128, HL * T], BF16, tag="o")

            def _phases():
                # ---- QKV projection ----
                # xT viewed as (p, chunk, t): row c*128+p -> [p, c, t]
                xT_v = xT.rearrange("(c p) t -> p c t", p=128)
                for w in range(NW):
                    xb = xpool.tile([128, NCHUNK, 512], BF16, tag="x")
                    nc.sync.dma_start(xb[:], xT_v[:, :, w * 512 : (w + 1) * 512])
                    # q and k: out (e-tile 128, t 512), e on partitions (transposed)
                    for grp, dst in ((0, q_sb), (1, k_sb)):
                        for j in range(HL):
                            ps = pspool.tile([128, 512], F32, tag="ps")
                            base = grp * 512 + j * 128
                            for c in range(NCHUNK):
                                nc.tensor.matmul(
                                    ps[:],
                                    w_sb[:, c, base : base + 128],
                                    xb[:, c, :],
                                    start=(c == 0),
                                    stop=(c == NCHUNK - 1),
                                )
                            nc.scalar.copy(
                                dst[:, j * T + w * 512 : j * T + (w + 1) * 512], ps[:]
                            )
                    # v: out (t-tile 128, e 512), natural layout
                    for ttl in range(4):
                        ttg = w * 4 + ttl
                        ps = pspool.tile([128, 512], F32, tag="ps")
                        for c in range(NCHUNK):
                            nc.tensor.matmul(
                                ps[:],
                                xb[:, c, ttl * 128 : (ttl + 1) * 128],
                                w_sb[:, c, 1024:1536],
                                start=(c == 0),
                                stop=(c == NCHUNK - 1),
                            )
                        nc.scalar.copy(v_sb[:, ttg * 512 : (ttg + 1) * 512], ps[:])

                # ---- RoPE on q, k (in place, d on partitions) ----
                for src in (q_sb, k_sb):
                    for h in range(HL):
                        sl = slice(h * T, (h + 1) * T)
                        rot = rotpool.tile([128, T], BF16, tag="rot")
                        nc.sync.dma_start(rot[0:64, :], src[64:128, sl])
                        nc.sync.dma_start(rot[64:128, :], src[0:64, sl])
                        nc.vector.tensor_mul(rot[:], rot[:], sin_sb[:])
                        nc.vector.tensor_mul(src[:, sl], src[:, sl], cos_sb[:])
                        nc.vector.tensor_add(src[:, sl], src[:, sl], rot[:])

                # ---- causal attention, head-by-head ----
                for h in range(HL):
                    hq = h * T
                    for w in range(NW):
                        nkt = 4 * w + 4
                        oT_ps = pspool.tile([128, 512], F32, tag="ps")
                        rs_ps = rspool.tile([1, 512], F32, tag="rs")
                        for kt in range(nkt):
                            if kt < 4 * w:
                                q0, n, diag = 512 * w, 512, False
                            else:
                                q0 = 128 * kt
                                n = 512 * (w + 1) - 128 * kt
                                diag = True
                            st = pspool.tile([128, 512], F32, tag="ps")
                            nc.tensor.matmul(
                                st[:, :n],
                                k_sb[:, hq + kt * 128 : hq + (kt + 1) * 128],
                                q_sb[:, hq + q0 : hq + q0 + n],
                                start=True,
                                stop=True,
                            )
                            pt = ptpool.tile([128, 512], BF16, tag="pt")
                            nc.scalar.activation(
                                pt[:, :n], st[:, :n], Exp, bias=0.0, scale=SCALE
                            )
                            if diag:
                                nc.vector.tensor_mul(pt[:, 0:128], pt[:, 0:128], tri_sb[:])
                            off = q0 - 512 * w
                            nc.tensor.matmul(
                                oT_ps[:, off:512],
                                v_sb[:, kt * 512 + h * 128 : kt * 512 + (h + 1) * 128],
                                pt[:, :n],
                                start=(kt == 0),
                                stop=(kt == nkt - 1),
                            )
                            nc.tensor.matmul(
                                rs_ps[:, off:512],
                                ones_sb[:],
                                pt[:, :n],
                                start=(kt == 0),
                                stop=(kt == nkt - 1),
                            )
                        rcp = rcppool.tile([1, 512], F32, tag="rcp")
                        nc.vector.reciprocal(rcp[:], rs_ps[:])
                        # broadcast rcp across partitions with a K=1 matmul
                        bc_ps = pspool.tile([128, 512], F32, tag="ps")
                        nc.tensor.matmul(
                            bc_ps[:], onesr_sb[:], rcp[:], start=True, stop=True
                        )
                        bc = bcpool.tile([128, 512], F32, tag="bc")
                        nc.vector.tensor_copy(bc[:], bc_ps[:])
                        nc.vector.tensor_mul(
                            o_sb[:, hq + w * 512 : hq + (w + 1) * 512], oT_ps[:], bc[:]
                        )

                # ---- output projection (partial over this core's heads) ----
                for tt in range(NTT):
                    for nw in range(DIM // 512):
                        yps = pspool.tile([128, 512], F32, tag="ps")
                        for hh in range(HL):
                            nc.tensor.matmul(
                                yps[:],
                                o_sb[:, hh * T + tt * 128 : hh * T + (tt + 1) * 128],
                                wp_sb[:, hh, nw * 512 : (nw + 1) * 512],
                                start=(hh == 0),
                                stop=(hh == HL - 1),
                            )
                        ysb = ypool.tile([128, 512], F32, tag="y")
                        nc.vector.tensor_copy(ysb[:], yps[:])
                        nc.sync.dma_start(
                            y[tt * 128 : (tt + 1) * 128, nw * 512 : (nw + 1) * 512], ysb[:]
                        )

            if loop_iters > 1:
                with tc.For_i(0, loop_iters, 1):
                    _phases()
            else:
                _phases()
    nc.compile()
    return nc


def _rope_tables():
    inv_freq = (
        1.0 / (10000.0 ** (np.arange(0, D, 2, dtype=np.float32) / np.float32(D)))
    ).astype(np.float32)
    tpos = np.arange(T, dtype=np.float32)
    freqs = tpos[:, None] * inv_freq[None, :]
    emb = np.concatenate([freqs, freqs], axis=1)  # (T, D)
    cos = np.cos(emb).astype(np.float32)
    sin = np.sin(emb).astype(np.float32)
    cosT = np.ascontiguousarray(cos.T)  # (D, T)
    sinTs = np.ascontiguousarray(sin.T)
    sinTs[0:64] *= -1.0  # fold rotate_half sign
    return cosT, sinTs


def make_in_maps(x, W_qkv, W_proj):
    cosT, sinTs = _rope_tables()
    tri = (np.arange(128)[None, :] >= np.arange(128)[:, None]).astype(
        ml_dtypes.bfloat16
    )
    tri = np.ascontiguousarray(tri)
    ones = np.ones((128, 1), dtype=ml_dtypes.bfloat16)
    in_maps = []
    for c in range(NCORES):
        b, g = divmod(c, 4)
        Wq = W_qkv[512 * g : 512 * (g + 1)]
        Wk = W_qkv[2048 + 512 * g : 2048 + 512 * (g + 1)]
        Wv = W_qkv[4096 + 512 * g : 4096 + 512 * (g + 1)]
        Wc = np.concatenate([Wq, Wk, Wv], axis=0)  # (1536, 2048)
        in_maps.append(
            {
                "xT": np.ascontiguousarray(x[b].T).astype(ml_dtypes.bfloat16),
                "wqkvT": np.ascontiguousarray(Wc.T).astype(ml_dtypes.bfloat16),
                "wpT": np.ascontiguousarray(
                    W_proj[:, 512 * g : 512 * (g + 1)].T
                ).astype(ml_dtypes.bfloat16),
                "cosT": cosT,
                "sinTs": sinTs,
                "tri": tri,
                "ones": ones,
                "ones_row": np.ones((1, 128), dtype=np.float32),
            }
        )
    return in_maps


def kernel(x, W_qkv, W_proj):
    global LAST_RESULTS
    x = np.asarray(x, dtype=np.float32)
    W_qkv = np.asarray(W_qkv, dtype=np.float32)
    W_proj = np.asarray(W_proj, dtype=np.float32)
    assert x.shape == (B, T, DIM) and W_qkv.shape == (3 * H * D, DIM)

    if "nc" not in _CACHE:
        _CACHE["nc"] = _build_module()
    nc = _CACHE["nc"]

    in_maps = make_in_maps(x, W_qkv, W_proj)
    trace = os.environ.get("KERNEL_TRACE", "0") == "1"
    res = bass_utils.run_bass_kernel_spmd(
        nc, in_maps, core_ids=list(range(NCORES)), trace=trace
    )
    LAST_RESULTS = res
    y = np.zeros((B, T, DIM), dtype=np.float32)
    for c in range(NCORES):
        y[c // 4] += res.results[c]["y"]
    return y



# revision 9
# speedup vs baseline: 1.3819x; 1.3819x over previous
"""Causal self-attention (B=2, T=2048, dim=2048, H=16, D=128) on 8 trn2 NeuronCores.

Sharding: data-parallel over batch (2 groups of 4 cores), tensor-parallel over
heads within a group (4 heads/core).  Each core computes its heads' QKV
projection (x @ Wqkv_part^T), RoPE, causal attention, and a partial output
projection against its W_proj column block; the host sums the 4 partials per
batch element.

v2 layout (all matmul operands bf16, fp32 accumulation), fused per 512-token
window w: QKV(w) -> RoPE(w) -> attention(all heads, w) -> proj(w-1):
  - x passed transposed (dim, T); q/k produced head-transposed (d, T) so
    S^T = kT.T @ qT directly; v natural (T, d).
  - RoPE rotate-half via a PE permutation matmul (swap matrix), sign folded
    into the sin table; cos/sin kept bf16.
  - scores pipelined depth-3: PE score matmul -> ScalarE exp (bf16) -> PE
    PV + rowsum-accumulate; rowsums in a single [1,512] PSUM tile at
    partition 0 (custom-DVE ops break on partition-offset slices).
  - softmax normalization entirely off the PE: reciprocal_approx_fast (DVE),
    partition_broadcast (Pool/GpSimd), fused multiply (DVE).
  - QKV/proj PSUM evacuation on DVE (Pool tensor_copy is not codegen-able),
    ScalarE stays pure-Exp (single act table load); y DMA'd per window.
"""

import os

import numpy as np
import ml_dtypes

import concourse.bass as bass
import concourse.bacc as bacc
import concourse.tile as tile
import concourse.mybir as mybir
from concourse import bass_utils

BF16 = mybir.dt.bfloat16
F32 = mybir.dt.float32

B, T, DIM = 2, 2048, 2048
H, D = 16, 128
HL = 4                   # heads per core
NCORES = 8
E = 3 * HL * D           # 1536 = per-core qkv output rows
NCHUNK = DIM // 128      # 16 contraction chunks
NW = T // 512            # 4 query windows
NTT = T // 128           # 16 token tiles
SCALE = 1.0 / float(np.sqrt(D))

_CACHE = {}
LAST_RESULTS = None


def _build_module():
    nc = bacc.Bacc("TRN2", target_bir_lowering=False, debug=False)
    xT = nc.dram_tensor("xT", (DIM, T), BF16, kind="ExternalInput")
    wqkvT = nc.dram_tensor("wqkvT", (DIM, E), BF16, kind="ExternalInput")
    wpT = nc.dram_tensor("wpT", (HL * D, DIM), BF16, kind="ExternalInput")
    cosT = nc.dram_tensor("cosT", (D, T), BF16, kind="ExternalInput")
    sinTs = nc.dram_tensor("sinTs", (D, T), BF16, kind="ExternalInput")
    tri = nc.dram_tensor("tri", (128, 128), BF16, kind="ExternalInput")
    swp = nc.dram_tensor("swp", (128, 128), BF16, kind="ExternalInput")
    ones = nc.dram_tensor("ones", (128, 1), BF16, kind="ExternalInput")
    y = nc.dram_tensor("y", (T, DIM), F32, kind="ExternalOutput")

    Exp = mybir.ActivationFunctionType.Exp

    with tile.TileContext(nc) as tc:
        with (
            tc.tile_pool(name="const", bufs=1) as cpool,
            tc.tile_pool(name="xp", bufs=2) as xpool,
            tc.tile_pool(name="rotp", bufs=3) as rotpool,
            tc.tile_pool(name="ptp", bufs=5) as ptpool,
            tc.tile_pool(name="rcpp", bufs=2) as rcppool,
            tc.tile_pool(name="bcp", bufs=2) as bcpool,
            tc.tile_pool(name="yp", bufs=3) as ypool,
            tc.tile_pool(name="psA", bufs=2, space="PSUM") as psA,
            tc.tile_pool(name="psS", bufs=3, space="PSUM") as psS,
            tc.tile_pool(name="psO", bufs=2, space="PSUM") as psO,
            tc.tile_pool(name="psR", bufs=1, space="PSUM") as psR,
        ):
            xT_v = xT.rearrange("(c p) t -> p c t", p=128)
            wqkv_v = wqkvT.rearrange("(c p) e -> p c e", p=128)

            w_sb = cpool.tile([128, NCHUNK, E], BF16, tag="w")
            wp_sb = cpool.tile([128, HL, DIM], BF16, tag="wp")
            cos_sb = cpool.tile([128, T], BF16, tag="cos")
            sin_sb = cpool.tile([128, T], BF16, tag="sin")
            tri_sb = cpool.tile([128, 128], BF16, tag="tri")
            swp_sb = cpool.tile([128, 128], BF16, tag="swp")
            ones_sb = cpool.tile([128, 1], BF16, tag="ones")
            q_sb = cpool.tile([128, HL * T], BF16, tag="q")
            k_sb = cpool.tile([128, HL * T], BF16, tag="k")
            v_sb = cpool.tile([128, NTT * HL * D], BF16, tag="v")
            o_sb = cpool.tile([128, HL * T], BF16, tag="o")

            xbs = {}

            def load_xb(w, split=False):
                xb = xpool.tile([128, NCHUNK, 512], BF16, tag="xb")
                sl = slice(w * 512, (w + 1) * 512)
                if split:
                    for g in range(4):
                        nc.sync.dma_start(
                            xb[:, 4 * g : 4 * g + 4, :], xT_v[:, 4 * g : 4 * g + 4, sl]
                        )
                else:
                    nc.sync.dma_start(xb[:], xT_v[:, :, sl])
                xbs[w] = xb

            # Startup DMAs, first-needed first.  Weights stream in E-column
            # slabs matching the qk-set processing order so window-0 compute
            # starts as soon as the first slab lands.
            load_xb(0, split=True)
            for s in range(4):
                nc.sync.dma_start(
                    w_sb[:, :, 256 * s : 256 * (s + 1)],
                    wqkv_v[:, :, 256 * s : 256 * (s + 1)],
                )
            nc.sync.dma_start(swp_sb[:], swp[:, :])
            nc.sync.dma_start(tri_sb[:], tri[:, :])
            nc.sync.dma_start(ones_sb[:], ones[:, :])
            nc.sync.dma_start(cos_sb[:], cosT[:, :])
            nc.sync.dma_start(sin_sb[:], sinTs[:, :])
            nc.sync.dma_start(w_sb[:, :, 1024:1536], wqkv_v[:, :, 1024:1536])
            load_xb(1)
            nc.sync.dma_start(wp_sb[:], wpT.rearrange("(h p) n -> p h n", p=128))

            def rope(src, h, w):
                sl = slice(h * T + w * 512, h * T + (w + 1) * 512)
                wsl = slice(w * 512, (w + 1) * 512)
                rp = psA.tile([128, 512], F32, tag="ps")
                nc.tensor.matmul(rp[:], swp_sb[:], src[:, sl], start=True, stop=True)
                rot = rotpool.tile([128, 512], BF16, tag="rot")
                nc.vector.tensor_mul(rot[:], rp[:], sin_sb[:, wsl])
                nc.vector.tensor_mul(src[:, sl], src[:, sl], cos_sb[:, wsl])
                nc.vector.tensor_add(src[:, sl], src[:, sl], rot[:])

            def qkv_window(w):
                xb = xbs[w]
                for grp, dst in ((0, q_sb), (1, k_sb)):
                    for j in range(HL):
                        ps = psA.tile([128, 512], F32, tag="ps")
                        base = grp * 512 + j * 128
                        for c in range(NCHUNK):
                            nc.tensor.matmul(
                                ps[:],
                                w_sb[:, c, base : base + 128],
                                xb[:, c, :],
                                start=(c == 0),
                                stop=(c == NCHUNK - 1),
                            )
                        sl = slice(j * T + w * 512, j * T + (w + 1) * 512)
                        nc.vector.tensor_copy(dst[:, sl], ps[:])
                # v sets with rope swaps interleaved (swap rhs reads the Pool
                # evacuation output, so give it a v-set of PE work as cover)
                swaps = [(q_sb, 0), (q_sb, 1), (q_sb, 2), (q_sb, 3),
                         (k_sb, 0), (k_sb, 1), (k_sb, 2), (k_sb, 3)]
                for ttl in range(4):
                    ttg = w * 4 + ttl
                    ps = psA.tile([128, 512], F32, tag="ps")
                    for c in range(NCHUNK):
                        nc.tensor.matmul(
                            ps[:],
                            xb[:, c, ttl * 128 : (ttl + 1) * 128],
                            w_sb[:, c, 1024:1536],
                            start=(c == 0),
                            stop=(c == NCHUNK - 1),
                        )
                    nc.vector.tensor_copy(v_sb[:, ttg * 512 : (ttg + 1) * 512], ps[:])
                    for src, h in swaps[2 * ttl : 2 * ttl + 2]:
                        rope(src, h, w)

            def attn_window(w):
                """All 4 heads of query-window w as one continuous kt-unit
                stream: the score/exp pipeline never drains across head
                boundaries, so the PE keeps ~DEPTH exp's in flight."""
                nkt = 4 * w + 4

                def geom(kt):
                    if kt < 4 * w:
                        return 512 * w, 512, False
                    q0 = 128 * kt
                    return q0, 512 * (w + 1) - 128 * kt, True

                def issue_score(u):
                    h, kt = divmod(u, nkt)
                    hq = h * T
                    q0, n, diag = geom(kt)
                    st = psS.tile([128, 512], F32, tag="st")
                    nc.tensor.matmul(
                        st[:, :n],
                        k_sb[:, hq + kt * 128 : hq + (kt + 1) * 128],
                        q_sb[:, hq + q0 : hq + q0 + n],
                        start=True,
                        stop=True,
                    )
                    pt = ptpool.tile([128, 512], BF16, tag="pt")
                    nc.scalar.activation(pt[:, :n], st[:, :n], Exp, bias=0.0, scale=SCALE)
                    if diag:
                        nc.vector.tensor_mul(pt[:, 0:128], pt[:, 0:128], tri_sb[:])
                    return pt, q0, n

                DEPTH = 3
                nu = HL * nkt
                pend = [issue_score(u) for u in range(min(DEPTH, nu))]
                oT = rs_row = None
                for u in range(nu):
                    h, kt = divmod(u, nkt)
                    hq = h * T
                    if kt == 0:
                        oT = psO.tile([128, 512], F32, tag="oT")
                        rs_row = psR.tile([1, 512], F32, tag="rs")
                    if u + DEPTH < nu:
                        pend.append(issue_score(u + DEPTH))
                    pt, q0, n = pend.pop(0)
                    off = q0 - 512 * w
                    nc.tensor.matmul(
                        oT[:, off:512],
                        v_sb[:, kt * 512 + h * 128 : kt * 512 + (h + 1) * 128],
                        pt[:, :n],
                        start=(kt == 0),
                        stop=(kt == nkt - 1),
                    )
                    nc.tensor.matmul(
                        rs_row[0:1, off:512],
                        ones_sb[:],
                        pt[:, :n],
                        start=(kt == 0),
                        stop=(kt == nkt - 1),
                    )
                    if kt == nkt - 1:
                        # softmax normalization: no PE involvement
                        rcp = rcppool.tile([1, 512], F32, tag="rcp")
                        nc.vector.reciprocal_approx_fast(rcp[:], rs_row[0:1, :])
                        bc = bcpool.tile([128, 512], F32, tag="bc")
                        nc.gpsimd.partition_broadcast(bc[:], rcp[:])
                        nc.vector.tensor_mul(
                            o_sb[:, hq + w * 512 : hq + (w + 1) * 512], oT[:], bc[:]
                        )

            def proj_window(w):
                for tt in range(4 * w, 4 * w + 4):
                    for nwi in range(DIM // 512):
                        yps = psA.tile([128, 512], F32, tag="ps")
                        for hh in range(HL):
                            nc.tensor.matmul(
                                yps[:],
                                o_sb[:, hh * T + tt * 128 : hh * T + (tt + 1) * 128],
                                wp_sb[:, hh, nwi * 512 : (nwi + 1) * 512],
                                start=(hh == 0),
                                stop=(hh == HL - 1),
                            )
                        ysb = ypool.tile([128, 512], F32, tag="ysb")
                        nc.scalar.copy(ysb[:], yps[:])
                        nc.sync.dma_start(
                            y[tt * 128 : (tt + 1) * 128, nwi * 512 : (nwi + 1) * 512],
                            ysb[:],
                        )

            for w in range(NW):
                if w + 2 < NW:
                    load_xb(w + 2)
                qkv_window(w)
                if w > 0:
                    proj_window(w - 1)
                attn_window(w)
            proj_window(NW - 1)

    nc.compile()
    return nc


def _rope_tables():
    inv_freq = (
        1.0 / (10000.0 ** (np.arange(0, D, 2, dtype=np.float32) / np.float32(D)))
    ).astype(np.float32)
    tpos = np.arange(T, dtype=np.float32)
    freqs = tpos[:, None] * inv_freq[None, :]
    emb = np.concatenate([freqs, freqs], axis=1)  # (T, D)
    cos = np.cos(emb).astype(np.float32)
    sin = np.sin(emb).astype(np.float32)
    cosT = np.ascontiguousarray(cos.T)  # (D, T)
    sinTs = np.ascontiguousarray(sin.T)
    sinTs[0:64] *= -1.0  # fold rotate_half sign
    return (
        cosT.astype(ml_dtypes.bfloat16),
        sinTs.astype(ml_dtypes.bfloat16),
    )


def make_in_maps(x, W_qkv, W_proj):
    cosT, sinTs = _rope_tables()
    tri = (np.arange(128)[None, :] >= np.arange(128)[:, None]).astype(
        ml_dtypes.bfloat16
    )
    tri = np.ascontiguousarray(tri)
    swp = np.zeros((128, 128), dtype=ml_dtypes.bfloat16)
    idx = np.arange(64)
    swp[idx + 64, idx] = 1.0
    swp[idx, idx + 64] = 1.0
    ones = np.ones((128, 1), dtype=ml_dtypes.bfloat16)
    in_maps = []
    for c in range(NCORES):
        b, g = divmod(c, 4)
        Wq = W_qkv[512 * g : 512 * (g + 1)]
        Wk = W_qkv[2048 + 512 * g : 2048 + 512 * (g + 1)]
        Wv = W_qkv[4096 + 512 * g : 4096 + 512 * (g + 1)]
        Wc = np.concatenate([Wq, Wk, Wv], axis=0)  # (1536, 2048)
        in_maps.append(
            {
                "xT": np.ascontiguousarray(x[b].T).astype(ml_dtypes.bfloat16),
                "wqkvT": np.ascontiguousarray(Wc.T).astype(ml_dtypes.bfloat16),
                "wpT": np.ascontiguousarray(
                    W_proj[:, 512 * g : 512 * (g + 1)].T
                ).astype(ml_dtypes.bfloat16),
                "cosT": cosT,
                "sinTs": sinTs,
                "tri": tri,
                "swp": swp,
                "ones": ones,
            }
        )
    return in_maps


def kernel(x, W_qkv, W_proj):
    global LAST_RESULTS
    x = np.asarray(x, dtype=np.float32)
    W_qkv = np.asarray(W_qkv, dtype=np.float32)
    W_proj = np.asarray(W_proj, dtype=np.float32)
    assert x.shape == (B, T, DIM) and W_qkv.shape == (3 * H * D, DIM)

    if "nc" not in _CACHE:
        _CACHE["nc"] = _build_module()
    nc = _CACHE["nc"]

    in_maps = make_in_maps(x, W_qkv, W_proj)
    trace = os.environ.get("KERNEL_TRACE", "0") == "1"
    res = bass_utils.run_bass_kernel_spmd(
        nc, in_maps, core_ids=list(range(NCORES)), trace=trace
    )
    LAST_RESULTS = res
    y = np.zeros((B, T, DIM), dtype=np.float32)
    for c in range(NCORES):
        y[c // 4] += res.results[c]["y"]
    return y


# revision 11
# speedup vs baseline: 1.4631x; 1.0587x over previous
"""Causal self-attention (B=2, T=2048, dim=2048, H=16, D=128) on 8 trn2 NeuronCores.

Sharding: data-parallel over batch (2 groups of 4 cores), tensor-parallel over
heads within a group (4 heads/core).  Each core computes its heads' QKV
projection (x @ Wqkv_part^T), RoPE, causal attention, and a partial output
projection against its W_proj column block; the host sums the 4 partials per
batch element.

v4 schedule (all matmul operands bf16, fp32 accumulation): software-pipelined
across 512-token windows — attention units of window w are WOVEN between the
QKV matmul sets of window w+1 and proj units of window w-1, so the ScalarE
exp stream (the attention-phase bottleneck) drains under QKV/proj PE work:
  - x passed transposed (dim, T); q/k produced head-transposed (d, T) so
    S^T = kT.T @ qT directly; v natural (T, d).
  - RoPE rotate-half via a PE permutation matmul (swap matrix), sign folded
    into the sin table; cos/sin kept bf16.
  - scores pipelined depth-3: PE score matmul -> ScalarE exp (bf16) -> PE
    PV + rowsum-accumulate; rowsum uses an all-ones [128,128] stationary so
    the row sums come out replicated across partitions (no broadcast needed).
  - softmax normalization off the PE: reciprocal_approx_fast + multiply (DVE).
  - QKV/proj PSUM evacuation on DVE; ScalarE stays pure-Exp (one act table
    load); weights stream in per-set E-column slabs; y DMA'd per window.
"""

import os

import numpy as np
import ml_dtypes

import concourse.bass as bass
import concourse.bacc as bacc
import concourse.tile as tile
import concourse.mybir as mybir
from concourse import bass_utils

BF16 = mybir.dt.bfloat16
F32 = mybir.dt.float32

B, T, DIM = 2, 2048, 2048
H, D = 16, 128
HL = 4                   # heads per core
NCORES = 8
E = 3 * HL * D           # 1536 = per-core qkv output rows
NCHUNK = DIM // 128      # 16 contraction chunks
NW = T // 512            # 4 query windows
NTT = T // 128           # 16 token tiles
SCALE = 1.0 / float(np.sqrt(D))

_CACHE = {}
LAST_RESULTS = None


def _build_module():
    nc = bacc.Bacc("TRN2", target_bir_lowering=False, debug=False)
    xT = nc.dram_tensor("xT", (DIM, T), BF16, kind="ExternalInput")
    wqkvT = nc.dram_tensor("wqkvT", (DIM, E), BF16, kind="ExternalInput")
    wpT = nc.dram_tensor("wpT", (HL * D, DIM), BF16, kind="ExternalInput")
    cosT = nc.dram_tensor("cosT", (D, T), BF16, kind="ExternalInput")
    sinTs = nc.dram_tensor("sinTs", (D, T), BF16, kind="ExternalInput")
    tri = nc.dram_tensor("tri", (128, 128), BF16, kind="ExternalInput")
    swp = nc.dram_tensor("swp", (128, 128), BF16, kind="ExternalInput")
    ones = nc.dram_tensor("ones", (128, 128), BF16, kind="ExternalInput")
    y = nc.dram_tensor("y", (T, DIM), F32, kind="ExternalOutput")

    Exp = mybir.ActivationFunctionType.Exp

    with tile.TileContext(nc) as tc:
        with (
            tc.tile_pool(name="const", bufs=1) as cpool,
            tc.tile_pool(name="xp", bufs=2) as xpool,
            tc.tile_pool(name="rotp", bufs=3) as rotpool,
            tc.tile_pool(name="ptp", bufs=5) as ptpool,
            tc.tile_pool(name="rcpp", bufs=2) as rcppool,
            tc.tile_pool(name="yp", bufs=3) as ypool,
            tc.tile_pool(name="psA", bufs=2, space="PSUM") as psA,
            tc.tile_pool(name="psS", bufs=3, space="PSUM") as psS,
            tc.tile_pool(name="psO", bufs=2, space="PSUM") as psO,
            tc.tile_pool(name="psR", bufs=1, space="PSUM") as psR,
        ):
            xT_v = xT.rearrange("(c p) t -> p c t", p=128)
            wqkv_v = wqkvT.rearrange("(c p) e -> p c e", p=128)

            w_sb = cpool.tile([128, NCHUNK, E], BF16, tag="w")
            wp_sb = cpool.tile([128, HL, DIM], BF16, tag="wp")
            cos_sb = cpool.tile([128, T], BF16, tag="cos")
            sin_sb = cpool.tile([128, T], BF16, tag="sin")
            tri_sb = cpool.tile([128, 128], BF16, tag="tri")
            swp_sb = cpool.tile([128, 128], BF16, tag="swp")
            ones_sb = cpool.tile([128, 128], BF16, tag="ones")
            q_sb = cpool.tile([128, HL * T], BF16, tag="q")
            k_sb = cpool.tile([128, HL * T], BF16, tag="k")
            v_sb = cpool.tile([128, NTT * HL * D], BF16, tag="v")
            o_sb = cpool.tile([128, HL * T], BF16, tag="o")

            xbs = {}

            def load_xb(w, split=False):
                xb = xpool.tile([128, NCHUNK, 512], BF16, tag="xb")
                sl = slice(w * 512, (w + 1) * 512)
                if split:
                    for g in range(4):
                        nc.sync.dma_start(
                            xb[:, 4 * g : 4 * g + 4, :], xT_v[:, 4 * g : 4 * g + 4, sl]
                        )
                else:
                    nc.sync.dma_start(xb[:], xT_v[:, :, sl])
                xbs[w] = xb

            # Startup DMAs, first-needed first: x window 0, then qk weight
            # slabs in the exact set order window-0 consumes them, then the
            # small tables, then the v slab.
            load_xb(0, split=True)
            for s in range(8):
                nc.sync.dma_start(
                    w_sb[:, :, 128 * s : 128 * (s + 1)],
                    wqkv_v[:, :, 128 * s : 128 * (s + 1)],
                )
            nc.sync.dma_start(swp_sb[:], swp[:, :])
            nc.sync.dma_start(tri_sb[:], tri[:, :])
            nc.sync.dma_start(ones_sb[:], ones[:, :])
            nc.sync.dma_start(cos_sb[:], cosT[:, :])
            nc.sync.dma_start(sin_sb[:], sinTs[:, :])
            nc.sync.dma_start(w_sb[:, :, 1024:1536], wqkv_v[:, :, 1024:1536])
            load_xb(1)
            nc.sync.dma_start(wp_sb[:], wpT.rearrange("(h p) n -> p h n", p=128))

            def rope(src, h, w):
                sl = slice(h * T + w * 512, h * T + (w + 1) * 512)
                wsl = slice(w * 512, (w + 1) * 512)
                rp = psA.tile([128, 512], F32, tag="ps")
                nc.tensor.matmul(rp[:], swp_sb[:], src[:, sl], start=True, stop=True)
                rot = rotpool.tile([128, 512], BF16, tag="rot")
                nc.vector.tensor_mul(rot[:], rp[:], sin_sb[:, wsl])
                nc.vector.tensor_mul(src[:, sl], src[:, sl], cos_sb[:, wsl])
                nc.vector.tensor_add(src[:, sl], src[:, sl], rot[:])

            def qkv_closures(w):
                """12 closures: 8 qk sets (each + its rope) then 4 v sets."""
                out = []

                def qk_set(grp, j):
                    def run():
                        xb = xbs[w]
                        dst = q_sb if grp == 0 else k_sb
                        ps = psA.tile([128, 512], F32, tag="ps")
                        base = grp * 512 + j * 128
                        for c in range(NCHUNK):
                            nc.tensor.matmul(
                                ps[:],
                                w_sb[:, c, base : base + 128],
                                xb[:, c, :],
                                start=(c == 0),
                                stop=(c == NCHUNK - 1),
                            )
                        sl = slice(j * T + w * 512, j * T + (w + 1) * 512)
                        nc.vector.tensor_copy(dst[:, sl], ps[:])
                        rope(dst, j, w)

                    return run

                def v_set(ttl):
                    def run():
                        xb = xbs[w]
                        ttg = w * 4 + ttl
                        ps = psA.tile([128, 512], F32, tag="ps")
                        for c in range(NCHUNK):
                            nc.tensor.matmul(
                                ps[:],
                                xb[:, c, ttl * 128 : (ttl + 1) * 128],
                                w_sb[:, c, 1024:1536],
                                start=(c == 0),
                                stop=(c == NCHUNK - 1),
                            )
                        nc.vector.tensor_copy(
                            v_sb[:, ttg * 512 : (ttg + 1) * 512], ps[:]
                        )

                    return run

                for grp in (0, 1):
                    for j in range(HL):
                        out.append(qk_set(grp, j))
                for ttl in range(4):
                    out.append(v_set(ttl))
                return out

            def attn_closures(w):
                """One closure per (head, key-tile) unit; the score/exp
                pipeline state is shared across the whole window stream."""
                nkt = 4 * w + 4
                nu = HL * nkt
                state = {"pend": [], "issued": 0, "oT": None, "rs": None}

                def geom(kt):
                    if kt < 4 * w:
                        return 512 * w, 512, False
                    q0 = 128 * kt
                    return q0, 512 * (w + 1) - 128 * kt, True

                def issue_score(u):
                    h, kt = divmod(u, nkt)
                    hq = h * T
                    q0, n, diag = geom(kt)
                    st = psS.tile([128, 512], F32, tag="st")
                    nc.tensor.matmul(
                        st[:, :n],
                        k_sb[:, hq + kt * 128 : hq + (kt + 1) * 128],
                        q_sb[:, hq + q0 : hq + q0 + n],
                        start=True,
                        stop=True,
                    )
                    pt = ptpool.tile([128, 512], BF16, tag="pt")
                    nc.scalar.activation(
                        pt[:, :n], st[:, :n], Exp, bias=0.0, scale=SCALE
                    )
                    if diag:
                        nc.vector.tensor_mul(pt[:, 0:128], pt[:, 0:128], tri_sb[:])
                    return pt, q0, n

                DEPTH = 3

                def unit(u):
                    def run():
                        h, kt = divmod(u, nkt)
                        hq = h * T
                        while state["issued"] < min(u + DEPTH + 1, nu):
                            state["pend"].append(issue_score(state["issued"]))
                            state["issued"] += 1
                        if kt == 0:
                            state["oT"] = psO.tile([128, 512], F32, tag="oT", name="oT")
                            state["rs"] = psR.tile([128, 512], F32, tag="rs", name="rs")
                        oT, rs = state["oT"], state["rs"]
                        pt, q0, n = state["pend"].pop(0)
                        off = q0 - 512 * w
                        nc.tensor.matmul(
                            oT[:, off:512],
                            v_sb[:, kt * 512 + h * 128 : kt * 512 + (h + 1) * 128],
                            pt[:, :n],
                            start=(kt == 0),
                            stop=(kt == nkt - 1),
                        )
                        nc.tensor.matmul(
                            rs[:, off:512],
                            ones_sb[:],
                            pt[:, :n],
                            start=(kt == 0),
                            stop=(kt == nkt - 1),
                        )
                        if kt == nkt - 1:
                            # softmax normalization: replicated row sums ->
                            # reciprocal + multiply, all off the PE
                            rcp = rcppool.tile([128, 512], F32, tag="rcp")
                            nc.vector.reciprocal_approx_fast(rcp[:], rs[:])
                            nc.vector.tensor_mul(
                                o_sb[:, hq + w * 512 : hq + (w + 1) * 512],
                                oT[:],
                                rcp[:],
                            )

                    return run

                return [unit(u) for u in range(nu)]

            def proj_closures(w):
                out = []

                def unit(tt, nwi):
                    def run():
                        yps = psA.tile([128, 512], F32, tag="ps")
                        for hh in range(HL):
                            nc.tensor.matmul(
                                yps[:],
                                o_sb[:, hh * T + tt * 128 : hh * T + (tt + 1) * 128],
                                wp_sb[:, hh, nwi * 512 : (nwi + 1) * 512],
                                start=(hh == 0),
                                stop=(hh == HL - 1),
                            )
                        ysb = ypool.tile([128, 512], F32, tag="ysb")
                        nc.vector.tensor_copy(ysb[:], yps[:])
                        nc.sync.dma_start(
                            y[tt * 128 : (tt + 1) * 128, nwi * 512 : (nwi + 1) * 512],
                            ysb[:],
                        )

                    return run

                for tt in range(4 * w, 4 * w + 4):
                    for nwi in range(DIM // 512):
                        out.append(unit(tt, nwi))
                return out

            def weave(big, small):
                """Emit big (QKV/proj) closures with small (attention) units
                distributed evenly between them."""
                if not big:
                    for s in small:
                        s()
                    return
                per = len(small) / len(big)
                si = 0.0
                done = 0
                for b in big:
                    b()
                    si += per
                    while done < int(si):
                        small[done]()
                        done += 1
                while done < len(small):
                    small[done]()
                    done += 1

            # Window 0's QKV runs unwoven (nothing to overlap yet), then each
            # steady-state iteration weaves attn(w) into qkv(w+1) + proj(w-1).
            for fn in qkv_closures(0):
                fn()
            for w in range(NW):
                if w + 2 < NW:
                    load_xb(w + 2)
                big = []
                if w + 1 < NW:
                    big += qkv_closures(w + 1)
                if w >= 1:
                    big += proj_closures(w - 1)
                weave(big, attn_closures(w))
            for fn in proj_closures(NW - 1):
                fn()

    nc.compile()
    return nc


def _rope_tables():
    inv_freq = (
        1.0 / (10000.0 ** (np.arange(0, D, 2, dtype=np.float32) / np.float32(D)))
    ).astype(np.float32)
    tpos = np.arange(T, dtype=np.float32)
    freqs = tpos[:, None] * inv_freq[None, :]
    emb = np.concatenate([freqs, freqs], axis=1)  # (T, D)
    cos = np.cos(emb).astype(np.float32)
    sin = np.sin(emb).astype(np.float32)
    cosT = np.ascontiguousarray(cos.T)  # (D, T)
    sinTs = np.ascontiguousarray(sin.T)
    sinTs[0:64] *= -1.0  # fold rotate_half sign
    return (
        cosT.astype(ml_dtypes.bfloat16),
        sinTs.astype(ml_dtypes.bfloat16),
    )


def make_in_maps(x, W_qkv, W_proj):
    cosT, sinTs = _rope_tables()
    tri = (np.arange(128)[None, :] >= np.arange(128)[:, None]).astype(
        ml_dtypes.bfloat16
    )
    tri = np.ascontiguousarray(tri)
    swp = np.zeros((128, 128), dtype=ml_dtypes.bfloat16)
    idx = np.arange(64)
    swp[idx + 64, idx] = 1.0
    swp[idx, idx + 64] = 1.0
    ones = np.ones((128, 128), dtype=ml_dtypes.bfloat16)
    in_maps = []
    for c in range(NCORES):
        b, g = divmod(c, 4)
        Wq = W_qkv[512 * g : 512 * (g + 1)]
        Wk = W_qkv[2048 + 512 * g : 2048 + 512 * (g + 1)]
        Wv = W_qkv[4096 + 512 * g : 4096 + 512 * (g + 1)]
        Wc = np.concatenate([Wq, Wk, Wv], axis=0)  # (1536, 2048)
        in_maps.append(
            {
                "xT": np.ascontiguousarray(x[b].T).astype(ml_dtypes.bfloat16),
                "wqkvT": np.ascontiguousarray(Wc.T).astype(ml_dtypes.bfloat16),
                "wpT": np.ascontiguousarray(
                    W_proj[:, 512 * g : 512 * (g + 1)].T
                ).astype(ml_dtypes.bfloat16),
                "cosT": cosT,
                "sinTs": sinTs,
                "tri": tri,
                "swp": swp,
                "ones": ones,
            }
        )
    return in_maps


def kernel(x, W_qkv, W_proj):
    global LAST_RESULTS
    x = np.asarray(x, dtype=np.float32)
    W_qkv = np.asarray(W_qkv, dtype=np.float32)
    W_proj = np.asarray(W_proj, dtype=np.float32)
    assert x.shape == (B, T, DIM) and W_qkv.shape == (3 * H * D, DIM)

    if "nc" not in _CACHE:
        _CACHE["nc"] = _build_module()
    nc = _CACHE["nc"]

    in_maps = make_in_maps(x, W_qkv, W_proj)
    trace = os.environ.get("KERNEL_TRACE", "0") == "1"
    res = bass_utils.run_bass_kernel_spmd(
        nc, in_maps, core_ids=list(range(NCORES)), trace=trace
    )
    LAST_RESULTS = res
    y = np.zeros((B, T, DIM), dtype=np.float32)
    for c in range(NCORES):
        y[c // 4] += res.results[c]["y"]
    return y
